# revision 54
# baseline (speedup 1.0000x reference)
"""LIF layer (T=64, B=128, 2048->2048) on 8 trn2 NeuronCores.

Strategy: hybrid shard = 2-way batch x 4-way out_dim. Core c handles
b-group (c % 2, 64 batches) and o-group (c // 2, 512 out channels).
Per core:
  GEMM  single-pass float32r (fp22-rounded inputs, 1 cyc/row on PE):
        cur[o, t*64+b] = sum_i W[o,i] * x[t,b,i], W stationary.
        Col-block schedule [512x6, 384, 384, 256]: the smooth descent
        keeps each block's PE time >= the prior block's scan time (no
        PSUM double-buffer handoff stall) and the 256 tail shrinks the
        exposed scan to 4 steps. 16 k-tiles x 4 o-tiles, PSUM block <=
        4 banks (one bank per o-tile group, ot innermost so consecutive
        matmuls alternate banks), double buffered. x and W arrive in
        k-chunks so matmuls overlap the startup stream.
  SCAN  64 sequential LIF steps on DVE, 3 fused ops per step:
        u = u*decay + cur (scalar_tensor_tensor, cur strided from PSUM)
        spk = u > thr     (tensor_tensor is_gt -> SBUF block tile)
        u -= spk          (tensor_tensor subtract; THR == 1.0)
Bias is folded away via u = mem - b/(1-decay): per-channel spike
threshold + init, zero per-step bias work.

DMA notes (measured): descriptor chunks must be >= 16KB/partition for
full queue rate; sync (SP) is the fast queue and carries x; W quarters
alternate gpsimd/scalar so consecutive k-ranges land in parallel.

Host-side prep: slice/transpose/pack x and W per core, final output is
a cheap transpose + concat. No collectives; pure SPMD.
"""

import math

import numpy as np

import concourse.bacc as bacc
import concourse.mybir as mybir
import concourse.tile as tile
from concourse import bass_utils

# Problem constants (hardcoded per contract)
T, B, I, O = 64, 128, 2048, 2048
N_CORES = 8
BG, OG = 2, 4              # batch groups x out groups
BL = B // BG               # batches per core
OL = O // OG               # out channels per core
NOT = OL // 128            # o-tiles
KT = I // 128              # 16 k-tiles
TBL = T * BL               # columns per core (t-major)
TPB = 8                    # timesteps per block
BLK = TPB * BL             # columns per block
N_BLK = TBL // BLK         # 8 blocks
TAU, THR = 2.0, 1.0
DECAY = math.exp(-1.0 / TAU)

F32 = mybir.dt.float32
F32R = mybir.dt.float32r
ALU = mybir.AluOpType

_cache = {}


# Block schedule: full blocks, then a smooth descent (384, 384, 256) at
# the end. The tail 256-col block shrinks the exposed scan tail to 4
# steps; the 384s keep each block's PE time >= the previous block's scan
# time so the PSUM double-buffer handoff never stalls (a hard 512->256
# step leaves PE waiting on the N-2 scan). Columns must stay >= 256 for
# f32r's 1 cyc/row; PE cost is width-independent (64 matmuls per block).
if BLK >= 512:
    COLS = [BLK] * (N_BLK - 2) + [384, 384, BLK // 2]
else:
    COLS = [BLK] * N_BLK
assert sum(COLS) == TBL


def _build_nc():
    nc = bacc.Bacc(trn_type="TRN2", target_bir_lowering=False)

    # DRAM I/O (all host-prepacked to the exact SBUF layouts). x is packed
    # block-major, each block [KT, ncols] flat per partition, so every DMA
    # is partition-contiguous (small chunks hit the ~7-30ns/descriptor
    # floor and cap a queue far below HBM speed).
    xp_d = nc.dram_tensor("xp", [128, KT * TBL], F32R, kind="ExternalInput")
    wp_d = nc.dram_tensor("wp", [128, KT, NOT, 128], F32R, kind="ExternalInput")
    # compact per-channel consts; broadcast over b on-chip via stride-0 APs
    thr_d = nc.dram_tensor("thr", [128, NOT], F32, kind="ExternalInput")
    u0_d = nc.dram_tensor("u0", [128, NOT], F32, kind="ExternalInput")
    out_d = nc.dram_tensor("out", [128, T, NOT, BL], F32, kind="ExternalOutput")

    with tile.TileContext(nc) as tc:
        with (
            tc.tile_pool(name="wpool", bufs=1) as wpool,
            tc.tile_pool(name="xpool", bufs=3) as xpool,
            tc.tile_pool(name="state", bufs=1) as state,
            tc.tile_pool(name="spkpool", bufs=4) as spkpool,
            tc.tile_pool(name="psum", bufs=2, space="PSUM") as psum_pool,
        ):
            u = state.tile([128, NOT, BL], F32)
            thr_s = state.tile([128, NOT], F32)
            u0_s = state.tile([128, NOT], F32)

            # W in quarters (8KB/partition chunks) so block 0's first
            # matmuls unlock as soon as the first k-quarter lands;
            # alternate queues so consecutive k-ranges land in parallel.
            # Sync carries only x (the fast queue).
            w_all = wpool.tile([128, KT, NOT, 128], F32R)
            w_splits = [(0, 4), (4, 8), (8, 12), (12, 16)]
            for i, (ka, kb) in enumerate(w_splits):
                eng = nc.gpsimd if i % 2 == 0 else nc.scalar
                eng.dma_start(w_all[:, ka:kb], wp_d[:, ka:kb])
            # consts after W on scalar: DVE only needs them at first scan
            nc.scalar.dma_start(u0_s[:], u0_d[:])
            nc.scalar.dma_start(thr_s[:], thr_d[:])
            thr_b = thr_s[:].unsqueeze(2).broadcast_to([128, NOT, BL])
            nc.vector.tensor_scalar_add(
                u[:], u0_s[:].unsqueeze(2).broadcast_to([128, NOT, BL]), 0.0)

            t0 = 0
            off = 0
            for bi, ncols in enumerate(COLS):
                tpb = ncols // BL
                # x tile, flat [KT, ncols] per partition
                xt = xpool.tile([128, KT * BLK], F32R, tag="xt")
                if bi == 0:
                    kcs = [range(0, 4), range(4, 8),
                           range(8, 12), range(12, 16)]
                elif ncols == BLK:
                    kcs = [range(0, KT // 2), range(KT // 2, KT)]
                else:
                    kcs = [range(0, KT)]
                for kc in kcs:
                    nc.sync.dma_start(
                        xt[:, kc.start * ncols:kc.stop * ncols],
                        xp_d[:, off + kc.start * ncols:off + kc.stop * ncols])

                # k-chunk outer, then k, then ot: matmuls start once the
                # first x/W chunk lands, and consecutive matmuls hit
                # DIFFERENT psum banks (ot cycles through 4 banks), so the
                # accumulate read-modify-write of one matmul never chases
                # the previous matmul's writes to the same addresses. All
                # NOT accumulation groups stay open across chunks — one
                # group per bank.
                ps = psum_pool.tile([128, NOT, BLK], F32, tag="ps")
                for kc in kcs:
                    for k in kc:
                        for ot in range(NOT):
                            nc.tensor.matmul(
                                ps[:, ot, :ncols],
                                w_all[:, k, ot],
                                xt[:, k * ncols:(k + 1) * ncols],
                                start=(k == 0),
                                stop=(k == KT - 1),
                            )

                # LIF steps consuming this block's PSUM
                spk = spkpool.tile([128, TPB, NOT, BL], F32, tag="spk")
                last_block = bi == len(COLS) - 1
                for tl in range(tpb):
                    cur = ps[:, :, tl * BL:(tl + 1) * BL]  # [128, NOT, BL]
                    nc.vector.scalar_tensor_tensor(
                        u[:], u[:], DECAY, cur, op0=ALU.mult, op1=ALU.add)
                    nc.vector.tensor_tensor(spk[:, tl], u[:], thr_b, op=ALU.is_gt)
                    if not (last_block and tl == tpb - 1):  # final u is dead
                        nc.vector.tensor_tensor(u[:], u[:], spk[:, tl], op=ALU.subtract)
                    if last_block:
                        # per-step out: the final piece waits only on the
                        # last is_gt instead of the whole block
                        nc.sync.dma_start(out_d[:, t0 + tl], spk[:, tl])
                if not last_block:
                    # Out rides both slow queues alternately; the
                    # next-to-last block uses sync (x is streamed by then).
                    if bi == len(COLS) - 2:
                        out_eng = nc.sync
                    else:
                        out_eng = nc.scalar if bi % 2 == 0 else nc.gpsimd
                    out_eng.dma_start(out_d[:, t0:t0 + tpb], spk[:, :tpb])
                t0 += tpb
                off += KT * ncols

    nc.compile()
    return nc


def _get_nc():
    if "nc" not in _cache:
        _cache["nc"] = _build_nc()
    return _cache["nc"]


def kernel(x_seq: np.ndarray, W: np.ndarray, b: np.ndarray) -> np.ndarray:
    nc = _get_nc()

    x_seq = np.ascontiguousarray(x_seq, dtype=np.float32)
    W = np.asarray(W, dtype=np.float32)
    b = np.asarray(b, dtype=np.float32)

    # x pack per b-group: [128(p), KT, TBL] with col = t*BL + b_local
    xps = []
    for bg in range(BG):
        xs = x_seq[:, bg * BL:(bg + 1) * BL, :].reshape(TBL, I)
        xpk = xs.reshape(TBL, KT, 128).transpose(2, 1, 0)   # [128, KT, TBL]
        parts, cs = [], 0
        for ncols in COLS:
            parts.append(xpk[:, :, cs:cs + ncols].reshape(128, KT * ncols))
            cs += ncols
        xps.append(np.ascontiguousarray(np.concatenate(parts, axis=1)))

    # W pack + folded bias tiles per o-group
    wps, thrs, u0s = [], [], []
    for og in range(OG):
        w_c = W[og * OL:(og + 1) * OL, :]                   # [OL, I]
        wps.append(np.ascontiguousarray(
            w_c.reshape(NOT, 128, KT, 128).transpose(3, 2, 0, 1)))
        b_c = b[og * OL:(og + 1) * OL]
        shift = b_c / (1.0 - DECAY)
        thrs.append(np.ascontiguousarray(
            (THR - shift).reshape(NOT, 128).T, dtype=np.float32))  # [128, NOT]
        u0s.append(np.ascontiguousarray(
            (-shift).reshape(NOT, 128).T, dtype=np.float32))

    in_maps = []
    for c in range(N_CORES):
        og, bg = c // BG, c % BG
        in_maps.append({
            "xp": xps[bg], "wp": wps[og], "thr": thrs[og], "u0": u0s[og],
        })

    res = bass_utils.run_bass_kernel_spmd(nc, in_maps, core_ids=list(range(N_CORES)))
    global LAST_RESULT
    LAST_RESULT = res

    # Assemble: out_c[of, t, ot, b] -> full[t, b, o]
    full = np.empty((T, B, O), dtype=np.float32)
    for c in range(N_CORES):
        og, bg = c // BG, c % BG
        oc = res.results[c]["out"]                          # [128, T, NOT, BL]
        full[:, bg * BL:(bg + 1) * BL, og * OL:(og + 1) * OL] = (
            oc.transpose(1, 3, 2, 0).reshape(T, BL, OL))
    return full


LAST_RESULT = None
